# revision 46
# baseline (speedup 1.0000x reference)
"""Trainium2 Bass kernel for the 2-layer GATv2 network (nn_GAT_49246095016405).

Sharding: destination-node partition across 8 cores. Edges live on the core
owning their dst, sorted by dst, padded to a uniform (blocks x chunks-per-block
x 128) structure. Per-edge work is done with PE matmuls in feature-major
layout via transposing dma_gather; segment softmax + scatter-add are done with
mask matmuls; node features are exchanged with HBM AllGathers (x for layer 1,
h for layer 2). Per-block work runs under tc.For_i hardware loops to keep the
program (and hence per-call compile/load cost) small; node tables use a
128-aligned padded per-core stride so every block is uniform.
"""
import math
import os
import numpy as np

import jax

# Persistent XLA compilation cache: the second run_bass_kernel_spmd call in a
# process (and any later process) deserializes the executable instead of
# re-running the full XLA+NEFF pipeline.
try:
    jax.config.update("jax_compilation_cache_dir", "/tmp/jax_cache")
    jax.config.update("jax_persistent_cache_min_entry_size_bytes", -1)
    jax.config.update("jax_persistent_cache_min_compile_time_secs", 0)
except Exception:
    pass

import concourse.bacc as bacc
import concourse.bass as bass
import concourse.mybir as mybir
import concourse.tile as tile
from concourse.bass import ds
from concourse.masks import make_identity
from concourse.bass_utils import run_bass_kernel_spmd

F16 = mybir.dt.float16
F32 = mybir.dt.float32
I16 = mybir.dt.int16
AF = mybir.ActivationFunctionType
OP = mybir.AluOpType

EPS = 1e-5


# ----------------------------------------------------------------------------
# device program
# ----------------------------------------------------------------------------

def build_gat(cfg):
    N, D, H = cfg["N"], cfg["D"], cfg["H"]
    CH1, CH2, CLASSES = cfg["CH1"], cfg["CH2"], cfg["CLASSES"]
    NC_, Nc, NB, CPB = cfg["n_cores"], cfg["Nc"], cfg["NB"], cfg["CPB"]
    D1 = H * CH1
    D2 = H * CH2
    CC1 = D1 // 128
    EB = CPB * 128
    EPAD = NB * EB
    GS = cfg.get("GS", 4)
    n_groups = math.ceil(CPB / GS)
    FC2 = D1 // 128
    NPC = NB * 128          # padded per-core node stride
    IW = EB // 16           # idx cols per block

    nc = bacc.Bacc("TRN2", num_devices=NC_)
    dp = nc.declare_dram_parameter

    # weight-slab layout (f16 [128, WCOLS], row-sharded 16 rows/core over the
    # on-device AllGather); row 0's tail columns carry the partition-1
    # vectors; woutT is f32 stored as 2 f16 columns per element (bitcast)
    WOFF = dict(wlT1=0, wrT1=512, wl2T=1024, wr2T=1536, attm1=2048,
                gl1=2056, gr1=2058, attm2=2060, ga2=2062,
                we1=2064, we2=2576, wa1=2704, wa2=2706, woutT=2708)
    WCOLS = 2708 + 2 * CLASSES
    # idxw columns: srcw | dstlw | dloc bytes (8 stripes of 16 rows, i16 view)
    IWC = EPAD // 16
    DLW = EPAD // 256          # i16 cols per 16-row stripe of dloc bytes

    RPW = math.ceil(128 / NC_)     # weight-slab rows per core
    # single packed param: x shard ([16, Nc*D/16]) | weight-slab shard |
    # idx tables (i16 bitcast) | dloc bytes (u8 bitcast)
    XW0 = Nc * D // 16
    XW1 = XW0 + WCOLS
    XW2 = XW1 + 2 * IWC
    XW3 = XW2 + 8 * DLW
    XWT = XW3 + EPAD // 16
    xw = dp("xw", [16, XWT], F16, isOutput=False)
    out_d = dp("out", [NPC, CLASSES], F16, isOutput=True)

    x16_loc = nc.dram_tensor("x16_loc", [NPC, D], F16)
    x16_full = nc.dram_tensor("x16_full", [NC_ * NPC, D], F16)
    x16_fsh = nc.dram_tensor("x16_fsh", [NC_ * NPC, D], F16,
                             addr_space="Shared")
    wbig_loc = nc.dram_tensor("wbig_loc", [RPW, WCOLS], F16)
    wbig_full = nc.dram_tensor("wbig_full", [NC_ * RPW, WCOLS], F16)
    wbig_fsh = nc.dram_tensor("wbig_fsh", [NC_ * RPW, WCOLS], F16,
                              addr_space="Shared")
    xl2_pad = nc.dram_tensor("xl2_pad", [NPC, D2], F16)
    xr2_d = nc.dram_tensor("xr2_d", [NPC, D2], F16)
    xl2_full = nc.dram_tensor("xl2_full", [NC_ * NPC, D2], F16)
    xl2_fsh = nc.dram_tensor("xl2_fsh", [NC_ * NPC, D2], F16,
                             addr_space="Shared")
    # DRAM staging for pre-gathered per-edge operands (dma_gather cannot run
    # inside hardware loops, so gathers run straight-line into these and the
    # block loops reload with plain DMAs).
    xsg = [nc.dram_tensor(f"xsg{l}", [128, EPAD], F16) for l in (1, 2)]
    xdg = [nc.dram_tensor(f"xdg{l}", [128, EPAD], F16) for l in (1, 2)]
    xeg = [nc.dram_tensor(f"xeg{l}", [128, EPAD // 128, 128], F16)
           for l in (1, 2)]

    with tile.TileContext(nc) as tc:
        with (
            tc.tile_pool(name="const", bufs=1) as cp,
            tc.tile_pool(name="persist", bufs=1) as pp,
            tc.tile_pool(name="sb", bufs=2) as sb,
            tc.tile_pool(name="gat", bufs=2) as gp,
            tc.tile_pool(name="ps", bufs=1, space="PSUM") as ps,
            tc.tile_pool(name="psT", bufs=2, space="PSUM") as psT,
        ):
            ident16 = cp.tile([128, 128], F16)
            ident32 = cp.tile([128, 128], F32)
            make_identity(nc, ident16[:])
            make_identity(nc, ident32[:])
            iota_i = cp.tile([128, 128], I16)
            nc.gpsimd.iota(iota_i[:], pattern=[[1, 128]], base=0,
                           channel_multiplier=0)
            iota16 = cp.tile([128, 128], F16)
            nc.vector.tensor_copy(iota16[:], iota_i[:])
            ones_col = cp.tile([128, 1], F16)
            nc.vector.memset(ones_col[:], 1.0)
            eps_col = cp.tile([128, 1], F32)
            nc.vector.memset(eps_col[:], EPS)

            # x and the weight slab arrive sharded; AllGather them on-device.
            nc.sync.dma_start(out=x16_loc[:Nc, :], in_=xw[:, :XW0])
            nc.gpsimd.collective_compute(
                "AllGather", OP.bypass,
                replica_groups=[list(range(NC_))],
                ins=[x16_loc[:]], outs=[x16_fsh[:]])
            nc.sync.dma_start(out=x16_full[:], in_=x16_fsh[:])

            nc.sync.dma_start(out=wbig_loc[:], in_=xw[:, XW0:XW1])
            nc.gpsimd.collective_compute(
                "AllGather", OP.bypass,
                replica_groups=[list(range(NC_))],
                ins=[wbig_loc[:]], outs=[wbig_fsh[:]])
            nc.sync.dma_start(out=wbig_full[:], in_=wbig_fsh[:])

            def load(t, dram):
                tt = cp.tile(list(dram.shape), dram.dtype, tag=t)
                nc.sync.dma_start(out=tt[:], in_=dram[:])
                return tt

            def wload(t, rows, cols):
                o = WOFF[t]
                tt = cp.tile([rows, cols], F16, tag=t)
                nc.sync.dma_start(out=tt[:], in_=wbig_full[:rows, o:o + cols])
                return tt

            # idx tables are uploaded once ([16, W]); gather wants the same
            # 16-partition wrap replicated in all 8 stripes of 128 partitions.
            def load_idx(t, col0, w):
                tt = cp.tile([128, w], I16, tag=t)
                for r in range(8):
                    nc.sync.dma_start(
                        out=tt[16 * r:16 * (r + 1), :],
                        in_=xw[:, XW1 + col0:XW1 + col0 + w].bitcast(I16))
                return tt

            srcw_s = load_idx("srcw", 0, IWC)
            dstlw_s = load_idx("dstlw", IWC, IWC)
            dloc8_s = cp.tile([128, EPAD // 128], mybir.dt.uint8,
                              tag="dloc8")
            for r in range(8):
                c0 = XW2 + r * DLW
                nc.sync.dma_start(
                    out=dloc8_s[16 * r:16 * (r + 1), :],
                    in_=xw[:, c0:c0 + DLW].bitcast(mybir.dt.uint8))
            dloc_s = cp.tile([128, EPAD // 128], F32, tag="dloc")
            nc.vector.tensor_copy(dloc_s[:], dloc8_s[:])
            eas_s = cp.tile([1, EPAD], F16, tag="eas")
            nc.sync.dma_start(out=eas_s[:],
                              in_=xw[:, XW3:XW3 + EPAD // 16])
            wlT1_s = wload("wlT1", D, D1)
            wrT1_s = wload("wrT1", D, D1)
            we1_s = wload("we1", 1, D1)
            attm1_s = wload("attm1", 128, CC1 * H)
            gl1_s = wload("gl1", D, H)
            gr1_s = wload("gr1", D, H)
            wa1_s = wload("wa1", 1, H)
            wl2T_s = wload("wl2T", 128, FC2 * D2)
            wr2T_s = wload("wr2T", 128, FC2 * D2)
            we2_s = wload("we2", 1, D2)
            attm2_s = wload("attm2", D2, H)
            ga2_s = wload("ga2", D2, H)
            wa2_s = wload("wa2", 1, H)
            woutT_s16 = wload("woutT", D2, 2 * CLASSES)

            def pregather(layer):
                if layer == 1:
                    gsrc, gdst, gem = x16_full, x16_loc, x16_full
                else:
                    gsrc, gdst, gem = xl2_full, xr2_d, xl2_full
                xsg_, xdg_, xeg_ = xsg[layer - 1], xdg[layer - 1], xeg[layer - 1]
                HB = EB // 2
                HI = IW // 2
                HC = CPB // 2
                for c in range(NB):
                    st = gp.tile([128, 1, EB], F16, tag="pgs")
                    dtt = gp.tile([128, 1, EB], F16, tag="pgd")
                    et = gp.tile([128, CPB, 128], F16, tag="pge")
                    if os.environ.get("GAT_NO_GATHER"):
                        nc.vector.memset(st[:], 0.25)
                        nc.vector.memset(dtt[:], 0.25)
                        nc.vector.memset(et[:], 0.25)
                    else:
                        for hf in range(2):
                            j0 = c * IW + hf * HI
                            nc.gpsimd.dma_gather(
                                out_ap=st[:, :, hf * HB:(hf + 1) * HB],
                                in_ap=gsrc[:],
                                idxs_ap=srcw_s[:, j0:j0 + HI],
                                num_idxs=HB, num_idxs_reg=HB, elem_size=128,
                                transpose=True)
                            nc.gpsimd.dma_gather(
                                out_ap=dtt[:, :, hf * HB:(hf + 1) * HB],
                                in_ap=gdst[:],
                                idxs_ap=dstlw_s[:, j0:j0 + HI],
                                num_idxs=HB, num_idxs_reg=HB, elem_size=128,
                                transpose=True)
                            nc.gpsimd.dma_gather(
                                out_ap=et[:, hf * HC:(hf + 1) * HC, :],
                                in_ap=gem[:],
                                idxs_ap=srcw_s[:, j0:j0 + HI],
                                num_idxs=HB, num_idxs_reg=HB, elem_size=128)
                    nc.sync.dma_start(out=xsg_[:, c * EB:(c + 1) * EB],
                                      in_=st[:, 0, :])
                    nc.sync.dma_start(out=xdg_[:, c * EB:(c + 1) * EB],
                                      in_=dtt[:, 0, :])
                    nc.sync.dma_start(out=xeg_[:, c * CPB:(c + 1) * CPB, :],
                                      in_=et[:])

            def edge_sweep(layer, b):
                cc_n = CC1 if layer == 1 else 1

                xsT = gp.tile([128, EB], F16, tag=f"xsT{layer}")
                xdT = gp.tile([128, EB], F16, tag=f"xdT{layer}")
                xem = gp.tile([128, CPB, 128], F16, tag=f"xem{layer}")
                nc.sync.dma_start(out=xsT[:],
                                  in_=xsg[layer - 1][:, ds(b * EB, EB)])
                nc.sync.dma_start(out=xdT[:],
                                  in_=xdg[layer - 1][:, ds(b * EB, EB)])
                nc.sync.dma_start(out=xem[:],
                                  in_=xeg[layer - 1][:, ds(b * CPB, CPB), :])

                if layer == 1:
                    aggT = ps.tile([128, H * 128], F32, tag="agg")
                    den = ps.tile([1, H * 128], F32, tag="den")
                else:
                    out2p = ps.tile([128, H * CH2], F32, tag="agg")
                    den2p = ps.tile([128, H], F32, tag="den")

                for g in range(n_groups):
                    k0 = g * GS
                    k1 = min(k0 + GS, CPB)
                    nk = k1 - k0
                    ew = nk * 128
                    es = slice(k0 * 128, k1 * 128)

                    lrT = gp.tile([128, cc_n * 512], F16, tag=f"lrT{layer}")
                    for cc in range(cc_n):
                        Tp = psT.tile([128, 512], F32, tag="T")
                        c0 = cc * 128
                        to = cc * 512
                        if layer == 1:
                            nc.tensor.matmul(Tp[:, :ew],
                                             wlT1_s[:, c0:c0 + 128],
                                             xsT[:, es],
                                             start=True, stop=False)
                            nc.tensor.matmul(Tp[:, :ew],
                                             wrT1_s[:, c0:c0 + 128],
                                             xdT[:, es],
                                             start=False, stop=False)
                            nc.tensor.matmul(
                                Tp[:, :ew], we1_s[:, c0:c0 + 128],
                                eas_s[:, ds(b * EB + k0 * 128, ew)],
                                start=False, stop=True)
                        else:
                            nc.tensor.matmul(Tp[:, :ew], ident16[:],
                                             xsT[:, es],
                                             start=True, stop=False)
                            nc.tensor.matmul(Tp[:, :ew], ident16[:],
                                             xdT[:, es],
                                             start=False, stop=False)
                            nc.tensor.matmul(
                                Tp[:, :ew], we2_s[:],
                                eas_s[:, ds(b * EB + k0 * 128, ew)],
                                start=False, stop=True)
                        on_act = (cc < cc_n // 2) if cc_n > 1 else (g % 2 == 0)
                        if on_act:
                            nc.scalar.activation(lrT[:, to:to + ew],
                                                 Tp[:, :ew], AF.Relu)
                        else:
                            nc.vector.tensor_scalar(lrT[:, to:to + ew],
                                                    Tp[:, :ew], 0.0, None,
                                                    OP.max)

                    lg = ps.tile([2, 512], F32, tag="lg")
                    for cc in range(cc_n):
                        am = (attm1_s[:, cc * H:(cc + 1) * H] if layer == 1
                              else attm2_s[:])
                        nc.tensor.matmul(lg[:, :ew], am,
                                         lrT[:, cc * 512:cc * 512 + ew],
                                         start=(cc == 0), stop=False)
                    glx = gl1_s if layer == 1 else ga2_s
                    grx = gr1_s if layer == 1 else ga2_s
                    wax = wa1_s if layer == 1 else wa2_s
                    nc.tensor.matmul(lg[:, :ew], glx[:], xsT[:, es],
                                     start=False, stop=False)
                    nc.tensor.matmul(lg[:, :ew], grx[:], xdT[:, es],
                                     start=False, stop=False)
                    nc.tensor.matmul(lg[:, :ew], wax[:],
                                     eas_s[:, ds(b * EB + k0 * 128, ew)],
                                     start=False, stop=True)

                    pT = gp.tile([2, 512], F16, tag=f"pT{layer}")
                    nc.scalar.activation(pT[:, :ew], lg[:, :ew], AF.Exp)

                    pE_p = ps.tile([128, 2 * GS], F16, tag="pEp")
                    for k in range(nk):
                        nc.tensor.transpose(
                            pE_p[:, 2 * k:2 * k + 2],
                            pT[:, k * 128:(k + 1) * 128], ident16[:2, :2])
                    pE = gp.tile([128, 2 * GS], F32, tag=f"pE{layer}")
                    nc.vector.tensor_copy(pE[:, :2 * nk], pE_p[:, :2 * nk])

                    for k in range(nk):
                        kk = k0 + k
                        first = kk == 0
                        last = kk == CPB - 1
                        A = gp.tile([128, H * 128], F16, tag=f"A{layer}")
                        for h in range(H):
                            nc.vector.tensor_scalar(
                                A[:, h * 128:(h + 1) * 128], iota16[:],
                                dloc_s[:, ds(b * CPB + kk, 1)],
                                pE[:, 2 * k + h:2 * k + h + 1],
                                OP.is_equal, OP.mult)
                        if layer == 1:
                            nc.tensor.matmul(aggT[:], xem[:, kk, :], A[:],
                                             start=first, stop=last)
                            nc.tensor.matmul(den[:], ones_col[:], A[:],
                                             start=first, stop=last)
                        else:
                            for h in range(H):
                                Ah = A[:, h * 128:(h + 1) * 128]
                                nc.tensor.matmul(
                                    out2p[:, h * CH2:(h + 1) * CH2], Ah,
                                    xem[:, kk, h * CH2:(h + 1) * CH2],
                                    start=(first and h == 0),
                                    stop=(last and h == H - 1))
                                nc.tensor.matmul(
                                    den2p[:, h:h + 1], Ah, ones_col[:],
                                    start=(first and h == 0),
                                    stop=(last and h == H - 1))

                if layer == 1:
                    aggT_sb = sb.tile([128, H * 128], F16, tag="aggT_sb")
                    nc.vector.tensor_copy(aggT_sb[:], aggT[:])
                    den_sb = sb.tile([1, H * 128], F32, tag="den_sb")
                    nc.vector.tensor_copy(den_sb[:], den[:])
                    den_t = ps.tile([128, H], F32, tag="finB")
                    for h in range(H):
                        nc.tensor.transpose(
                            den_t[:, h:h + 1],
                            den_sb[:, h * 128:(h + 1) * 128], ident32[:1, :1])
                    rc1 = sb.tile([128, H], F32, tag="rc1")
                    nc.vector.reciprocal(rc1[:], den_t[:])

                    o1p = ps.tile([128, D1], F32, tag="finA")
                    for h in range(H):
                        nc.tensor.matmul(
                            o1p[:, h * CH1:(h + 1) * CH1],
                            aggT_sb[:, h * 128:(h + 1) * 128],
                            wlT1_s[:, h * CH1:(h + 1) * CH1],
                            start=True, stop=True)
                    o_sb = sb.tile([128, D1], F16, tag="o_sb")
                    for h in range(H):
                        nc.vector.tensor_scalar(
                            o_sb[:, h * CH1:(h + 1) * CH1],
                            o1p[:, h * CH1:(h + 1) * CH1],
                            rc1[:, h:h + 1], None, OP.mult)
                    t_sb = sb.tile([128, D1], F16, tag="t_sb")
                    nc.scalar.activation(t_sb[:], o_sb[:], AF.Tanh, scale=0.5)
                    nc.vector.tensor_scalar(t_sb[:], t_sb[:], 1.0, 0.5,
                                            OP.add, OP.mult)
                    h1loc = sb.tile([128, D1], F16, tag="h1loc")
                    nc.vector.tensor_tensor(h1loc[:], o_sb[:], t_sb[:],
                                            OP.mult)
                    sq = sb.tile([128, D1], F16, tag="sq")
                    ms1b = ps.tile([128, 1], F32, tag="agg")
                    nc.scalar.activation(sq[:], h1loc[:], AF.Square,
                                         accum_out=ms1b[:])
                    rs1b = sb.tile([128, 1], F32, tag="rs1b")
                    nc.scalar.activation(rs1b[:], ms1b[:], AF.Sqrt,
                                         scale=1.0 / D1, bias=eps_col[:])
                    nc.vector.reciprocal(rs1b[:], rs1b[:])
                    h1T_p = ps.tile([128, D1], F16, tag="finA")
                    for fc in range(FC2):
                        nc.tensor.transpose(
                            h1T_p[:, fc * 128:(fc + 1) * 128],
                            h1loc[:, fc * 128:(fc + 1) * 128],
                            ident16[:])
                    h1T = sb.tile([128, D1], F16, tag="h1T")
                    nc.vector.tensor_copy(h1T[:], h1T_p[:])
                    xl2p = ps.tile([128, D2], F32, tag="finA")
                    xr2p = ps.tile([128, D2], F32, tag="finB")
                    for fc in range(FC2):
                        nc.tensor.matmul(xl2p[:],
                                         h1T[:, fc * 128:(fc + 1) * 128],
                                         wl2T_s[:, fc * D2:(fc + 1) * D2],
                                         start=(fc == 0),
                                         stop=(fc == FC2 - 1))
                        nc.tensor.matmul(xr2p[:],
                                         h1T[:, fc * 128:(fc + 1) * 128],
                                         wr2T_s[:, fc * D2:(fc + 1) * D2],
                                         start=(fc == 0),
                                         stop=(fc == FC2 - 1))
                    xl2_sb = sb.tile([128, D2], F16, tag="xl2_sb")
                    xr2_sb = sb.tile([128, D2], F16, tag="xr2_sb")
                    nc.vector.tensor_scalar(xl2_sb[:], xl2p[:], rs1b[:],
                                            None, OP.mult)
                    nc.vector.tensor_scalar(xr2_sb[:], xr2p[:], rs1b[:],
                                            None, OP.mult)
                    nc.sync.dma_start(out=xl2_pad[ds(b * 128, 128), :],
                                      in_=xl2_sb[:])
                    nc.sync.dma_start(out=xr2_d[ds(b * 128, 128), :],
                                      in_=xr2_sb[:])
                else:
                    rc2 = sb.tile([128, H], F32, tag="rc2")
                    nc.vector.reciprocal(rc2[:], den2p[:])
                    h2loc = sb.tile([128, D2], F32, tag="h2loc")
                    for h in range(H):
                        nc.vector.tensor_scalar(
                            h2loc[:, h * CH2:(h + 1) * CH2],
                            out2p[:, h * CH2:(h + 1) * CH2],
                            rc2[:, h:h + 1], None, OP.mult)
                    sq2 = sb.tile([128, D2], F32, tag="sq2")
                    ms2b = ps.tile([128, 1], F32, tag="agg")
                    nc.scalar.activation(sq2[:], h2loc[:], AF.Square,
                                         accum_out=ms2b[:])
                    rs2b = sb.tile([128, 1], F32, tag="rs2b")
                    nc.scalar.activation(rs2b[:], ms2b[:], AF.Sqrt,
                                         scale=1.0 / D2, bias=eps_col[:])
                    nc.vector.reciprocal(rs2b[:], rs2b[:])
                    h2n = sb.tile([128, D2], F32, tag="h2n")
                    nc.vector.tensor_scalar(h2n[:], h2loc[:], rs2b[:],
                                            None, OP.mult)
                    h2nT_p = ps.tile([128, D2], F32, tag="finA")
                    nc.tensor.transpose(h2nT_p[:], h2n[:], ident32[:])
                    h2nT = sb.tile([128, D2], F32, tag="h2nT")
                    nc.vector.tensor_copy(h2nT[:], h2nT_p[:])
                    op_p = ps.tile([128, CLASSES], F32, tag="finB")
                    nc.tensor.matmul(op_p[:], h2nT[:],
                                     woutT_s16[:].bitcast(F32),
                                     start=True, stop=True)
                    o_fin = sb.tile([128, CLASSES], F16, tag="o_fin")
                    nc.vector.tensor_copy(o_fin[:], op_p[:])
                    nc.sync.dma_start(out=out_d[ds(b * 128, 128), :],
                                      in_=o_fin[:])

            # ================= layer 1 =================
            pregather(1)
            with tc.For_i(0, NB, 1, name="l1") as b:
                edge_sweep(1, b)

            if not os.environ.get("GAT_NO_CC"):
                nc.gpsimd.collective_compute(
                    "AllGather", OP.bypass,
                    replica_groups=[list(range(NC_))],
                    ins=[xl2_pad[:]], outs=[xl2_fsh[:]])
                nc.sync.dma_start(out=xl2_full[:], in_=xl2_fsh[:])

            # ================= layer 2 =================
            pregather(2)
            with tc.For_i(0, NB, 1, name="l2") as b:
                edge_sweep(2, b)

    nc.finalize()
    return nc


# ----------------------------------------------------------------------------
# host side
# ----------------------------------------------------------------------------

def _wrap16(v):
    return np.ascontiguousarray(v.reshape(-1, 16).T)


def prep_core(cfg, src, dst, ea, k):
    Nc, NB, CPB, NPC = cfg["Nc"], cfg["NB"], cfg["CPB"], cfg["NPC"]
    EB = CPB * 128
    EPAD = NB * EB
    ldst = dst - k * Nc
    order = np.argsort(ldst, kind="stable")
    src, ea, ldst = src[order], ea[order], ldst[order]
    blk = ldst // 128
    # global node ids in the 128-aligned padded layout
    srcp = (src // Nc) * NPC + src % Nc

    sg = np.zeros(EPAD, np.int16)
    dl = np.zeros(EPAD, np.int16)
    dloc = np.full(EPAD, 255.0, np.float32)
    eap = np.zeros(EPAD, np.float32)
    starts = np.searchsorted(blk, np.arange(NB))
    slots = blk * EB + (np.arange(len(blk)) - starts[blk])
    sg[slots] = srcp
    dl[slots] = ldst
    dloc[slots] = ldst - blk * 128
    eap[slots] = ea
    # ghost dst slots for the ragged last block, mapped onto its pad edges
    nb_last = Nc - (NB - 1) * 128
    if nb_last < 128:
        n_last = len(blk) - starts[NB - 1]
        ng = 128 - nb_last
        assert n_last + ng <= EB
        o = (NB - 1) * EB + n_last
        dloc[o:o + ng] = np.arange(nb_last, 128)
    return sg, dl, dloc, eap


def make_cfg_and_maps(x, ei, ea, weights, n_cores=8, gs=4):
    N, D = x.shape
    H = 2
    (Wl1, Wr1, We1, att1, Wl2, Wr2, We2, att2,
     w_ln1, w_ln3, W_out) = weights
    D1, D2 = Wl1.shape[0], Wl2.shape[0]
    CH1, CH2 = D1 // H, D2 // H
    CLASSES = W_out.shape[0]
    Nc = N // n_cores

    src, dst = ei[0].astype(np.int64), ei[1].astype(np.int64)
    cnt = np.bincount(dst, minlength=N).astype(np.float32)
    ssum = np.bincount(dst, weights=ea, minlength=N).astype(np.float32)
    loop_attr = ssum / np.maximum(cnt, 1.0)
    src = np.concatenate([src, np.arange(N)])
    dst = np.concatenate([dst, np.arange(N)])
    ea2 = np.concatenate([ea, loop_attr])

    NB = math.ceil(Nc / 128)
    NPC = NB * 128
    core = dst // Nc
    maxe = 0
    for k in range(n_cores):
        m = core == k
        ld = dst[m] - k * Nc
        bc = np.bincount(ld // 128, minlength=NB).astype(np.int64)
        nb_last = Nc - (NB - 1) * 128
        bc[NB - 1] += 128 - nb_last
        maxe = max(maxe, int(bc.max()))
    CPB = max(2, 2 * math.ceil(maxe / 256))   # even, for half-block gathers

    cfg = dict(N=N, D=D, H=H, CH1=CH1, CH2=CH2, CLASSES=CLASSES,
               n_cores=n_cores, Nc=Nc, NB=NB, CPB=CPB, GS=gs, NPC=NPC)

    f16 = np.float16
    CC1 = D1 // 128
    attf1 = att1.reshape(D1)
    attm1 = np.zeros((128, CC1 * H), np.float32)
    for j in range(D1):
        h = j // CH1
        attm1[j % 128, (j // 128) * H + h] = 0.8 * attf1[j]
    gl1 = np.zeros((D, H), np.float32)
    gr1 = np.zeros((D, H), np.float32)
    for h in range(H):
        sl = slice(h * CH1, (h + 1) * CH1)
        gl1[:, h] = 0.2 * (Wl1[sl, :].T @ att1[h])
        gr1[:, h] = 0.2 * (Wr1[sl, :].T @ att1[h])
    wa1 = np.array([[0.2 * float(We1[h * CH1:(h + 1) * CH1, 0] @ att1[h])
                     for h in range(H)]], np.float32)
    FC2 = D1 // 128
    Wl2f = Wl2 * w_ln1[None, :]
    Wr2f = Wr2 * w_ln1[None, :]
    wl2T = np.ascontiguousarray(
        Wl2f.T.reshape(FC2, 128, D2).transpose(1, 0, 2)).reshape(128, FC2 * D2)
    wr2T = np.ascontiguousarray(
        Wr2f.T.reshape(FC2, 128, D2).transpose(1, 0, 2)).reshape(128, FC2 * D2)
    attf2 = att2.reshape(D2)
    attm2 = np.zeros((D2, H), np.float32)
    ga2 = np.zeros((D2, H), np.float32)
    for j in range(D2):
        h = j // CH2
        attm2[j, h] = 0.8 * attf2[j]
        ga2[j, h] = 0.2 * attf2[j]
    wa2 = np.array([[0.2 * float(We2[h * CH2:(h + 1) * CH2, 0] @ att2[h])
                     for h in range(H)]], np.float32)
    woutT = np.ascontiguousarray((W_out * w_ln3[None, :]).T).astype(np.float32)

    x16 = x.astype(f16)

    # pack every weight into one [128, WCOLS] slab (row 0's tail columns
    # hold the partition-1 vectors; woutT f32 as f16-pair columns); must
    # mirror WOFF in build_gat.
    WCOLS = 2708 + 2 * CLASSES
    wbig = np.zeros((128, WCOLS), f16)
    wbig[:, 0:512] = np.ascontiguousarray(Wl1.T).astype(f16)
    wbig[:, 512:1024] = np.ascontiguousarray(Wr1.T).astype(f16)
    wbig[:, 1024:1536] = wl2T.astype(f16)
    wbig[:, 1536:2048] = wr2T.astype(f16)
    wbig[:, 2048:2056] = attm1.astype(f16)
    wbig[:, 2056:2058] = gl1.astype(f16)
    wbig[:, 2058:2060] = gr1.astype(f16)
    wbig[:, 2060:2062] = attm2.astype(f16)
    wbig[:, 2062:2064] = ga2.astype(f16)
    wbig[0, 2064:2576] = np.ascontiguousarray(We1.T).astype(f16)[0]
    wbig[0, 2576:2704] = np.ascontiguousarray(We2.T).astype(f16)[0]
    wbig[0, 2704:2706] = wa1.astype(f16)[0]
    wbig[0, 2706:2708] = wa2.astype(f16)[0]
    wbig[:, 2708:] = np.ascontiguousarray(woutT).view(f16)

    RPW = math.ceil(128 / n_cores)
    wbig_pad = np.zeros((n_cores * RPW, WCOLS), f16)
    wbig_pad[:128] = wbig

    in_maps = []
    for k in range(n_cores):
        m = core == k
        sg, dl, dloc, eap = prep_core(cfg, src[m], dst[m], ea2[m], k)
        dloc8 = np.ascontiguousarray(
            dloc.reshape(-1, 128).T).astype(np.uint8)
        # dloc bytes ride in idxw as 8 stripes of 16 rows, viewed as i16
        dloc_i16 = np.concatenate(
            [np.ascontiguousarray(dloc8[16 * r:16 * (r + 1)]).view(np.int16)
             for r in range(8)], axis=1)
        idx_all = np.concatenate([_wrap16(sg), _wrap16(dl), dloc_i16],
                                 axis=1)
        xw = np.concatenate(
            [x16[k * Nc:(k + 1) * Nc].reshape(16, -1),
             wbig_pad[RPW * k:RPW * (k + 1)],
             idx_all.view(f16),
             eap.astype(f16).reshape(16, -1)], axis=1)
        in_maps.append({"xw": np.ascontiguousarray(xw)})
    return cfg, in_maps


_NC_CACHE = {}


def _build_cached(cfg):
    key = tuple(sorted(cfg.items()))
    if key not in _NC_CACHE:
        _NC_CACHE[key] = build_gat(cfg)
    return _NC_CACHE[key]


def kernel(**inputs):
    x = np.asarray(inputs["x"], np.float32)
    ei = np.asarray(inputs["edge_index"])
    ea = np.asarray(inputs["edge_attr"], np.float32)[:, 0]
    weights = tuple(np.asarray(inputs[k], np.float32) for k in
                    ("Wl1", "Wr1", "We1", "att1", "Wl2", "Wr2", "We2", "att2",
                     "w_ln1", "w_ln3", "W_out"))
    cfg, in_maps = make_cfg_and_maps(x, ei, ea, weights)
    nc = _build_cached(cfg)
    res = run_bass_kernel_spmd(nc, in_maps, list(range(cfg["n_cores"])))
    out = np.concatenate([res.results[k]["out"][:cfg["Nc"]]
                          for k in range(cfg["n_cores"])], axis=0)
    return np.asarray(out, np.float32)


if __name__ == "__main__":
    import reference as ref
    inputs = {k: np.asarray(v) for k, v in ref.setup_inputs().items()}
    got = kernel(**inputs)
    exp = np.asarray(ref.reference(**inputs))
    rel = np.abs(got - exp).max() / np.abs(exp).max()
    print(f"Relative error: {rel:.3e}")


# revision 47
# speedup vs baseline: 1.0410x; 1.0410x over previous
"""Trainium2 Bass kernel for the 2-layer GATv2 network (nn_GAT_49246095016405).

Sharding: destination-node partition across 8 cores. Edges live on the core
owning their dst, sorted by dst, padded to a uniform (blocks x chunks-per-block
x 128) structure. Per-edge work is done with PE matmuls in feature-major
layout via transposing dma_gather; segment softmax + scatter-add are done with
mask matmuls; node features are exchanged with HBM AllGathers (x for layer 1,
h for layer 2). Per-block work runs under tc.For_i hardware loops to keep the
program (and hence per-call compile/load cost) small; node tables use a
128-aligned padded per-core stride so every block is uniform.
"""
import math
import os
import numpy as np

import jax

# Persistent XLA compilation cache: the second run_bass_kernel_spmd call in a
# process (and any later process) deserializes the executable instead of
# re-running the full XLA+NEFF pipeline.
try:
    jax.config.update("jax_compilation_cache_dir", "/tmp/jax_cache")
    jax.config.update("jax_persistent_cache_min_entry_size_bytes", -1)
    jax.config.update("jax_persistent_cache_min_compile_time_secs", 0)
except Exception:
    pass

import concourse.bacc as bacc
import concourse.bass as bass
import concourse.mybir as mybir
import concourse.tile as tile
from concourse.bass import ds
from concourse.masks import make_identity
from concourse.bass_utils import run_bass_kernel_spmd

F16 = mybir.dt.float16
F32 = mybir.dt.float32
I16 = mybir.dt.int16
AF = mybir.ActivationFunctionType
OP = mybir.AluOpType

EPS = 1e-5


# ----------------------------------------------------------------------------
# device program
# ----------------------------------------------------------------------------

def build_gat(cfg):
    N, D, H = cfg["N"], cfg["D"], cfg["H"]
    CH1, CH2, CLASSES = cfg["CH1"], cfg["CH2"], cfg["CLASSES"]
    NC_, Nc, NB, CPB = cfg["n_cores"], cfg["Nc"], cfg["NB"], cfg["CPB"]
    D1 = H * CH1
    D2 = H * CH2
    CC1 = D1 // 128
    EB = CPB * 128
    EPAD = NB * EB
    GS = cfg.get("GS", 4)
    n_groups = math.ceil(CPB / GS)
    FC2 = D1 // 128
    NPC = NB * 128          # padded per-core node stride
    IW = EB // 16           # idx cols per block

    nc = bacc.Bacc("TRN2", num_devices=NC_)
    dp = nc.declare_dram_parameter

    # weight-slab layout (f16 [128, WCOLS], row-sharded 16 rows/core over the
    # on-device AllGather); row 0's tail columns carry the partition-1
    # vectors; woutT is f32 stored as 2 f16 columns per element (bitcast)
    WOFF = dict(wlT1=0, wrT1=512, wl2T=1024, wr2T=1536, attm1=2048,
                gl1=2056, gr1=2058, attm2=2060, ga2=2062,
                we1=2064, we2=2576, wa1=2704, wa2=2706, woutT=2708)
    WCOLS = 2708 + 2 * CLASSES
    # idxw columns: srcw | dstlw | dloc bytes (8 stripes of 16 rows, i16 view)
    IWC = EPAD // 16
    DLW = EPAD // 256          # i16 cols per 16-row stripe of dloc bytes

    RPW = math.ceil(128 / NC_)     # weight-slab rows per core
    # single packed param: x shard ([16, Nc*D/16]) | weight-slab shard |
    # idx tables (i16 bitcast) | dloc bytes (u8 bitcast)
    XW0 = Nc * D // 16
    XW1 = XW0 + WCOLS
    XW2 = XW1 + 2 * IWC
    XW3 = XW2 + 8 * DLW
    XWT = XW3 + EPAD // 16
    xw = dp("xw", [16, XWT], F16, isOutput=False)
    out_d = dp("out", [NPC, CLASSES], F16, isOutput=True)

    x16_loc = nc.dram_tensor("x16_loc", [NPC, D], F16)
    x16_full = nc.dram_tensor("x16_full", [NC_ * NPC, D], F16)
    x16_fsh = nc.dram_tensor("x16_fsh", [NC_ * NPC, D], F16,
                             addr_space="Shared")
    wbig_loc = nc.dram_tensor("wbig_loc", [RPW, WCOLS], F16)
    wbig_full = nc.dram_tensor("wbig_full", [NC_ * RPW, WCOLS], F16)
    wbig_fsh = nc.dram_tensor("wbig_fsh", [NC_ * RPW, WCOLS], F16,
                              addr_space="Shared")
    xl2_pad = nc.dram_tensor("xl2_pad", [NPC, D2], F16)
    xr2_d = nc.dram_tensor("xr2_d", [NPC, D2], F16)
    xl2_full = nc.dram_tensor("xl2_full", [NC_ * NPC, D2], F16)
    xl2_fsh = nc.dram_tensor("xl2_fsh", [NC_ * NPC, D2], F16,
                             addr_space="Shared")
    # DRAM staging for pre-gathered per-edge operands (dma_gather cannot run
    # inside hardware loops, so gathers run straight-line into these and the
    # block loops reload with plain DMAs).
    xsg = [nc.dram_tensor(f"xsg{l}", [128, EPAD], F16) for l in (1, 2)]
    xdg = [nc.dram_tensor(f"xdg{l}", [128, EPAD], F16) for l in (1, 2)]
    xeg = [nc.dram_tensor(f"xeg{l}", [128, EPAD // 128, 128], F16)
           for l in (1, 2)]

    with tile.TileContext(nc) as tc:
        with (
            tc.tile_pool(name="const", bufs=1) as cp,
            tc.tile_pool(name="sb", bufs=2) as sb,
            tc.tile_pool(name="gat", bufs=2) as gp,
            tc.tile_pool(name="ps", bufs=1, space="PSUM") as ps,
            tc.tile_pool(name="psT", bufs=2, space="PSUM") as psT,
        ):
            ident16 = cp.tile([128, 128], F16)
            ident32 = cp.tile([128, 128], F32)
            make_identity(nc, ident16[:])
            make_identity(nc, ident32[:])
            iota_i = cp.tile([128, 128], I16)
            nc.gpsimd.iota(iota_i[:], pattern=[[1, 128]], base=0,
                           channel_multiplier=0)
            iota16 = cp.tile([128, 128], F16)
            nc.vector.tensor_copy(iota16[:], iota_i[:])
            ones_col = cp.tile([128, 1], F16)
            nc.vector.memset(ones_col[:], 1.0)
            eps_col = cp.tile([128, 1], F32)
            nc.vector.memset(eps_col[:], EPS)

            # x and the weight slab arrive sharded; AllGather them on-device.
            nc.sync.dma_start(out=x16_loc[:Nc, :], in_=xw[:, :XW0])
            nc.gpsimd.collective_compute(
                "AllGather", OP.bypass,
                replica_groups=[list(range(NC_))],
                ins=[x16_loc[:]], outs=[x16_fsh[:]])
            nc.sync.dma_start(out=x16_full[:], in_=x16_fsh[:])

            nc.sync.dma_start(out=wbig_loc[:], in_=xw[:, XW0:XW1])
            nc.gpsimd.collective_compute(
                "AllGather", OP.bypass,
                replica_groups=[list(range(NC_))],
                ins=[wbig_loc[:]], outs=[wbig_fsh[:]])
            nc.sync.dma_start(out=wbig_full[:], in_=wbig_fsh[:])

            def wload(t, rows, cols):
                o = WOFF[t]
                tt = cp.tile([rows, cols], F16, tag=t)
                nc.sync.dma_start(out=tt[:], in_=wbig_full[:rows, o:o + cols])
                return tt

            # idx tables are uploaded once ([16, W]); gather wants the same
            # 16-partition wrap replicated in all 8 stripes of 128 partitions.
            def load_idx(t, col0, w):
                tt = cp.tile([128, w], I16, tag=t)
                for r in range(8):
                    nc.sync.dma_start(
                        out=tt[16 * r:16 * (r + 1), :],
                        in_=xw[:, XW1 + col0:XW1 + col0 + w].bitcast(I16))
                return tt

            srcw_s = load_idx("srcw", 0, IWC)
            dstlw_s = load_idx("dstlw", IWC, IWC)
            dloc8_s = cp.tile([128, EPAD // 128], mybir.dt.uint8,
                              tag="dloc8")
            for r in range(8):
                c0 = XW2 + r * DLW
                nc.sync.dma_start(
                    out=dloc8_s[16 * r:16 * (r + 1), :],
                    in_=xw[:, c0:c0 + DLW].bitcast(mybir.dt.uint8))
            dloc_s = cp.tile([128, EPAD // 128], F32, tag="dloc")
            nc.vector.tensor_copy(dloc_s[:], dloc8_s[:])
            eas_s = cp.tile([1, EPAD], F16, tag="eas")
            nc.sync.dma_start(out=eas_s[:],
                              in_=xw[:, XW3:XW3 + EPAD // 16])
            wlT1_s = wload("wlT1", D, D1)
            wrT1_s = wload("wrT1", D, D1)
            we1_s = wload("we1", 1, D1)
            attm1_s = wload("attm1", 128, CC1 * H)
            gl1_s = wload("gl1", D, H)
            gr1_s = wload("gr1", D, H)
            wa1_s = wload("wa1", 1, H)
            wl2T_s = wload("wl2T", 128, FC2 * D2)
            wr2T_s = wload("wr2T", 128, FC2 * D2)
            we2_s = wload("we2", 1, D2)
            attm2_s = wload("attm2", D2, H)
            ga2_s = wload("ga2", D2, H)
            wa2_s = wload("wa2", 1, H)
            woutT_s16 = wload("woutT", D2, 2 * CLASSES)

            def pregather(layer):
                if layer == 1:
                    gsrc, gdst, gem = x16_full, x16_loc, x16_full
                else:
                    gsrc, gdst, gem = xl2_full, xr2_d, xl2_full
                xsg_, xdg_, xeg_ = xsg[layer - 1], xdg[layer - 1], xeg[layer - 1]
                HB = EB // 2
                HI = IW // 2
                HC = CPB // 2
                for c in range(NB):
                    st = gp.tile([128, 1, EB], F16, tag="pgs")
                    dtt = gp.tile([128, 1, EB], F16, tag="pgd")
                    et = gp.tile([128, CPB, 128], F16, tag="pge")
                    if os.environ.get("GAT_NO_GATHER"):
                        nc.vector.memset(st[:], 0.25)
                        nc.vector.memset(dtt[:], 0.25)
                        nc.vector.memset(et[:], 0.25)
                    else:
                        for hf in range(2):
                            j0 = c * IW + hf * HI
                            nc.gpsimd.dma_gather(
                                out_ap=st[:, :, hf * HB:(hf + 1) * HB],
                                in_ap=gsrc[:],
                                idxs_ap=srcw_s[:, j0:j0 + HI],
                                num_idxs=HB, num_idxs_reg=HB, elem_size=128,
                                transpose=True)
                            nc.gpsimd.dma_gather(
                                out_ap=dtt[:, :, hf * HB:(hf + 1) * HB],
                                in_ap=gdst[:],
                                idxs_ap=dstlw_s[:, j0:j0 + HI],
                                num_idxs=HB, num_idxs_reg=HB, elem_size=128,
                                transpose=True)
                            nc.gpsimd.dma_gather(
                                out_ap=et[:, hf * HC:(hf + 1) * HC, :],
                                in_ap=gem[:],
                                idxs_ap=srcw_s[:, j0:j0 + HI],
                                num_idxs=HB, num_idxs_reg=HB, elem_size=128)
                    nc.sync.dma_start(out=xsg_[:, c * EB:(c + 1) * EB],
                                      in_=st[:, 0, :])
                    nc.sync.dma_start(out=xdg_[:, c * EB:(c + 1) * EB],
                                      in_=dtt[:, 0, :])
                    nc.sync.dma_start(out=xeg_[:, c * CPB:(c + 1) * CPB, :],
                                      in_=et[:])

            def edge_sweep(layer, b):
                cc_n = CC1 if layer == 1 else 1

                xsT = gp.tile([128, EB], F16, tag=f"xsT{layer}")
                xdT = gp.tile([128, EB], F16, tag=f"xdT{layer}")
                xem = gp.tile([128, CPB, 128], F16, tag=f"xem{layer}")
                nc.sync.dma_start(out=xsT[:],
                                  in_=xsg[layer - 1][:, ds(b * EB, EB)])
                nc.sync.dma_start(out=xdT[:],
                                  in_=xdg[layer - 1][:, ds(b * EB, EB)])
                nc.sync.dma_start(out=xem[:],
                                  in_=xeg[layer - 1][:, ds(b * CPB, CPB), :])

                if layer == 1:
                    aggT = ps.tile([128, H * 128], F32, tag="agg")
                    den = ps.tile([1, H * 128], F32, tag="den")
                else:
                    out2p = ps.tile([128, H * CH2], F32, tag="agg")
                    den2p = ps.tile([128, H], F32, tag="den")

                for g in range(n_groups):
                    k0 = g * GS
                    k1 = min(k0 + GS, CPB)
                    nk = k1 - k0
                    ew = nk * 128
                    es = slice(k0 * 128, k1 * 128)

                    lrT = gp.tile([128, cc_n * 512], F16, tag=f"lrT{layer}")
                    for cc in range(cc_n):
                        Tp = psT.tile([128, 512], F32, tag="T")
                        c0 = cc * 128
                        to = cc * 512
                        if layer == 1:
                            nc.tensor.matmul(Tp[:, :ew],
                                             wlT1_s[:, c0:c0 + 128],
                                             xsT[:, es],
                                             start=True, stop=False)
                            nc.tensor.matmul(Tp[:, :ew],
                                             wrT1_s[:, c0:c0 + 128],
                                             xdT[:, es],
                                             start=False, stop=False)
                            nc.tensor.matmul(
                                Tp[:, :ew], we1_s[:, c0:c0 + 128],
                                eas_s[:, ds(b * EB + k0 * 128, ew)],
                                start=False, stop=True)
                        else:
                            nc.tensor.matmul(Tp[:, :ew], ident16[:],
                                             xsT[:, es],
                                             start=True, stop=False)
                            nc.tensor.matmul(Tp[:, :ew], ident16[:],
                                             xdT[:, es],
                                             start=False, stop=False)
                            nc.tensor.matmul(
                                Tp[:, :ew], we2_s[:],
                                eas_s[:, ds(b * EB + k0 * 128, ew)],
                                start=False, stop=True)
                        on_act = (cc < cc_n // 2) if cc_n > 1 else (g % 2 == 0)
                        if on_act:
                            nc.scalar.activation(lrT[:, to:to + ew],
                                                 Tp[:, :ew], AF.Relu)
                        else:
                            nc.vector.tensor_scalar(lrT[:, to:to + ew],
                                                    Tp[:, :ew], 0.0, None,
                                                    OP.max)

                    lg = ps.tile([2, 512], F32, tag="lg")
                    for cc in range(cc_n):
                        am = (attm1_s[:, cc * H:(cc + 1) * H] if layer == 1
                              else attm2_s[:])
                        nc.tensor.matmul(lg[:, :ew], am,
                                         lrT[:, cc * 512:cc * 512 + ew],
                                         start=(cc == 0), stop=False)
                    glx = gl1_s if layer == 1 else ga2_s
                    grx = gr1_s if layer == 1 else ga2_s
                    wax = wa1_s if layer == 1 else wa2_s
                    nc.tensor.matmul(lg[:, :ew], glx[:], xsT[:, es],
                                     start=False, stop=False)
                    nc.tensor.matmul(lg[:, :ew], grx[:], xdT[:, es],
                                     start=False, stop=False)
                    nc.tensor.matmul(lg[:, :ew], wax[:],
                                     eas_s[:, ds(b * EB + k0 * 128, ew)],
                                     start=False, stop=True)

                    pT = gp.tile([2, 512], F16, tag=f"pT{layer}")
                    nc.scalar.activation(pT[:, :ew], lg[:, :ew], AF.Exp)

                    pE_p = ps.tile([128, 2 * GS], F16, tag="pEp")
                    for k in range(nk):
                        nc.tensor.transpose(
                            pE_p[:, 2 * k:2 * k + 2],
                            pT[:, k * 128:(k + 1) * 128], ident16[:2, :2])
                    pE = gp.tile([128, 2 * GS], F32, tag=f"pE{layer}")
                    nc.vector.tensor_copy(pE[:, :2 * nk], pE_p[:, :2 * nk])

                    for k in range(nk):
                        kk = k0 + k
                        first = kk == 0
                        last = kk == CPB - 1
                        A = gp.tile([128, H * 128], F16, tag=f"A{layer}")
                        for h in range(H):
                            nc.vector.tensor_scalar(
                                A[:, h * 128:(h + 1) * 128], iota16[:],
                                dloc_s[:, ds(b * CPB + kk, 1)],
                                pE[:, 2 * k + h:2 * k + h + 1],
                                OP.is_equal, OP.mult)
                        if layer == 1:
                            nc.tensor.matmul(aggT[:], xem[:, kk, :], A[:],
                                             start=first, stop=last)
                            nc.tensor.matmul(den[:], ones_col[:], A[:],
                                             start=first, stop=last)
                        else:
                            for h in range(H):
                                Ah = A[:, h * 128:(h + 1) * 128]
                                nc.tensor.matmul(
                                    out2p[:, h * CH2:(h + 1) * CH2], Ah,
                                    xem[:, kk, h * CH2:(h + 1) * CH2],
                                    start=(first and h == 0),
                                    stop=(last and h == H - 1))
                                nc.tensor.matmul(
                                    den2p[:, h:h + 1], Ah, ones_col[:],
                                    start=(first and h == 0),
                                    stop=(last and h == H - 1))

                if layer == 1:
                    aggT_sb = sb.tile([128, H * 128], F16, tag="aggT_sb")
                    nc.vector.tensor_copy(aggT_sb[:], aggT[:])
                    den_sb = sb.tile([1, H * 128], F32, tag="den_sb")
                    nc.vector.tensor_copy(den_sb[:], den[:])
                    den_t = ps.tile([128, H], F32, tag="finB")
                    for h in range(H):
                        nc.tensor.transpose(
                            den_t[:, h:h + 1],
                            den_sb[:, h * 128:(h + 1) * 128], ident32[:1, :1])
                    rc1 = sb.tile([128, H], F32, tag="rc1")
                    nc.vector.reciprocal(rc1[:], den_t[:])

                    o1p = ps.tile([128, D1], F32, tag="finA")
                    for h in range(H):
                        nc.tensor.matmul(
                            o1p[:, h * CH1:(h + 1) * CH1],
                            aggT_sb[:, h * 128:(h + 1) * 128],
                            wlT1_s[:, h * CH1:(h + 1) * CH1],
                            start=True, stop=True)
                    o_sb = sb.tile([128, D1], F16, tag="o_sb")
                    for h in range(H):
                        nc.vector.tensor_scalar(
                            o_sb[:, h * CH1:(h + 1) * CH1],
                            o1p[:, h * CH1:(h + 1) * CH1],
                            rc1[:, h:h + 1], None, OP.mult)
                    t_sb = sb.tile([128, D1], F16, tag="t_sb")
                    nc.scalar.activation(t_sb[:], o_sb[:], AF.Tanh, scale=0.5)
                    nc.vector.tensor_scalar(t_sb[:], t_sb[:], 1.0, 0.5,
                                            OP.add, OP.mult)
                    h1loc = sb.tile([128, D1], F16, tag="h1loc")
                    nc.vector.tensor_tensor(h1loc[:], o_sb[:], t_sb[:],
                                            OP.mult)
                    sq = sb.tile([128, D1], F16, tag="sq")
                    ms1b = ps.tile([128, 1], F32, tag="agg")
                    nc.scalar.activation(sq[:], h1loc[:], AF.Square,
                                         accum_out=ms1b[:])
                    rs1b = sb.tile([128, 1], F32, tag="rs1b")
                    nc.scalar.activation(rs1b[:], ms1b[:], AF.Sqrt,
                                         scale=1.0 / D1, bias=eps_col[:])
                    nc.vector.reciprocal(rs1b[:], rs1b[:])
                    h1T_p = ps.tile([128, D1], F16, tag="finA")
                    for fc in range(FC2):
                        nc.tensor.transpose(
                            h1T_p[:, fc * 128:(fc + 1) * 128],
                            h1loc[:, fc * 128:(fc + 1) * 128],
                            ident16[:])
                    h1T = sb.tile([128, D1], F16, tag="h1T")
                    nc.vector.tensor_copy(h1T[:], h1T_p[:])
                    xl2p = ps.tile([128, D2], F32, tag="finA")
                    xr2p = ps.tile([128, D2], F32, tag="finB")
                    for fc in range(FC2):
                        nc.tensor.matmul(xl2p[:],
                                         h1T[:, fc * 128:(fc + 1) * 128],
                                         wl2T_s[:, fc * D2:(fc + 1) * D2],
                                         start=(fc == 0),
                                         stop=(fc == FC2 - 1))
                        nc.tensor.matmul(xr2p[:],
                                         h1T[:, fc * 128:(fc + 1) * 128],
                                         wr2T_s[:, fc * D2:(fc + 1) * D2],
                                         start=(fc == 0),
                                         stop=(fc == FC2 - 1))
                    xl2_sb = sb.tile([128, D2], F16, tag="xl2_sb")
                    xr2_sb = sb.tile([128, D2], F16, tag="xr2_sb")
                    nc.vector.tensor_scalar(xl2_sb[:], xl2p[:], rs1b[:],
                                            None, OP.mult)
                    nc.vector.tensor_scalar(xr2_sb[:], xr2p[:], rs1b[:],
                                            None, OP.mult)
                    nc.sync.dma_start(out=xl2_pad[ds(b * 128, 128), :],
                                      in_=xl2_sb[:])
                    nc.sync.dma_start(out=xr2_d[ds(b * 128, 128), :],
                                      in_=xr2_sb[:])
                else:
                    rc2 = sb.tile([128, H], F32, tag="rc2")
                    nc.vector.reciprocal(rc2[:], den2p[:])
                    h2loc = sb.tile([128, D2], F32, tag="h2loc")
                    for h in range(H):
                        nc.vector.tensor_scalar(
                            h2loc[:, h * CH2:(h + 1) * CH2],
                            out2p[:, h * CH2:(h + 1) * CH2],
                            rc2[:, h:h + 1], None, OP.mult)
                    sq2 = sb.tile([128, D2], F32, tag="sq2")
                    ms2b = ps.tile([128, 1], F32, tag="agg")
                    nc.scalar.activation(sq2[:], h2loc[:], AF.Square,
                                         accum_out=ms2b[:])
                    rs2b = sb.tile([128, 1], F32, tag="rs2b")
                    nc.scalar.activation(rs2b[:], ms2b[:], AF.Sqrt,
                                         scale=1.0 / D2, bias=eps_col[:])
                    nc.vector.reciprocal(rs2b[:], rs2b[:])
                    h2n = sb.tile([128, D2], F32, tag="h2n")
                    nc.vector.tensor_scalar(h2n[:], h2loc[:], rs2b[:],
                                            None, OP.mult)
                    h2nT_p = ps.tile([128, D2], F32, tag="finA")
                    nc.tensor.transpose(h2nT_p[:], h2n[:], ident32[:])
                    h2nT = sb.tile([128, D2], F32, tag="h2nT")
                    nc.vector.tensor_copy(h2nT[:], h2nT_p[:])
                    op_p = ps.tile([128, CLASSES], F32, tag="finB")
                    nc.tensor.matmul(op_p[:], h2nT[:],
                                     woutT_s16[:].bitcast(F32),
                                     start=True, stop=True)
                    o_fin = sb.tile([128, CLASSES], F16, tag="o_fin")
                    nc.vector.tensor_copy(o_fin[:], op_p[:])
                    nc.sync.dma_start(out=out_d[ds(b * 128, 128), :],
                                      in_=o_fin[:])

            # ================= layer 1 =================
            pregather(1)
            with tc.For_i(0, NB, 1, name="l1") as b:
                edge_sweep(1, b)

            if not os.environ.get("GAT_NO_CC"):
                nc.gpsimd.collective_compute(
                    "AllGather", OP.bypass,
                    replica_groups=[list(range(NC_))],
                    ins=[xl2_pad[:]], outs=[xl2_fsh[:]])
                nc.sync.dma_start(out=xl2_full[:], in_=xl2_fsh[:])

            # ================= layer 2 =================
            pregather(2)
            with tc.For_i(0, NB, 1, name="l2") as b:
                edge_sweep(2, b)

    nc.finalize()
    return nc


# ----------------------------------------------------------------------------
# host side
# ----------------------------------------------------------------------------

def _wrap16(v):
    return np.ascontiguousarray(v.reshape(-1, 16).T)


def prep_core(cfg, src, dst, ea, k):
    Nc, NB, CPB, NPC = cfg["Nc"], cfg["NB"], cfg["CPB"], cfg["NPC"]
    EB = CPB * 128
    EPAD = NB * EB
    ldst = dst - k * Nc
    order = np.argsort(ldst, kind="stable")
    src, ea, ldst = src[order], ea[order], ldst[order]
    blk = ldst // 128
    # global node ids in the 128-aligned padded layout
    srcp = (src // Nc) * NPC + src % Nc

    sg = np.zeros(EPAD, np.int16)
    dl = np.zeros(EPAD, np.int16)
    dloc = np.full(EPAD, 255.0, np.float32)
    eap = np.zeros(EPAD, np.float32)
    starts = np.searchsorted(blk, np.arange(NB))
    slots = blk * EB + (np.arange(len(blk)) - starts[blk])
    sg[slots] = srcp
    dl[slots] = ldst
    dloc[slots] = ldst - blk * 128
    eap[slots] = ea
    # ghost dst slots for the ragged last block, mapped onto its pad edges
    nb_last = Nc - (NB - 1) * 128
    if nb_last < 128:
        n_last = len(blk) - starts[NB - 1]
        ng = 128 - nb_last
        assert n_last + ng <= EB
        o = (NB - 1) * EB + n_last
        dloc[o:o + ng] = np.arange(nb_last, 128)
    return sg, dl, dloc, eap


def make_cfg_and_maps(x, ei, ea, weights, n_cores=8, gs=4):
    N, D = x.shape
    H = 2
    (Wl1, Wr1, We1, att1, Wl2, Wr2, We2, att2,
     w_ln1, w_ln3, W_out) = weights
    D1, D2 = Wl1.shape[0], Wl2.shape[0]
    CH1, CH2 = D1 // H, D2 // H
    CLASSES = W_out.shape[0]
    Nc = N // n_cores

    src, dst = ei[0].astype(np.int64), ei[1].astype(np.int64)
    cnt = np.bincount(dst, minlength=N).astype(np.float32)
    ssum = np.bincount(dst, weights=ea, minlength=N).astype(np.float32)
    loop_attr = ssum / np.maximum(cnt, 1.0)
    src = np.concatenate([src, np.arange(N)])
    dst = np.concatenate([dst, np.arange(N)])
    ea2 = np.concatenate([ea, loop_attr])

    NB = math.ceil(Nc / 128)
    NPC = NB * 128
    core = dst // Nc
    maxe = 0
    for k in range(n_cores):
        m = core == k
        ld = dst[m] - k * Nc
        bc = np.bincount(ld // 128, minlength=NB).astype(np.int64)
        nb_last = Nc - (NB - 1) * 128
        bc[NB - 1] += 128 - nb_last
        maxe = max(maxe, int(bc.max()))
    CPB = max(2, 2 * math.ceil(maxe / 256))   # even, for half-block gathers

    cfg = dict(N=N, D=D, H=H, CH1=CH1, CH2=CH2, CLASSES=CLASSES,
               n_cores=n_cores, Nc=Nc, NB=NB, CPB=CPB, GS=gs, NPC=NPC)

    f16 = np.float16
    CC1 = D1 // 128
    attf1 = att1.reshape(D1)
    attm1 = np.zeros((128, CC1 * H), np.float32)
    for j in range(D1):
        h = j // CH1
        attm1[j % 128, (j // 128) * H + h] = 0.8 * attf1[j]
    gl1 = np.zeros((D, H), np.float32)
    gr1 = np.zeros((D, H), np.float32)
    for h in range(H):
        sl = slice(h * CH1, (h + 1) * CH1)
        gl1[:, h] = 0.2 * (Wl1[sl, :].T @ att1[h])
        gr1[:, h] = 0.2 * (Wr1[sl, :].T @ att1[h])
    wa1 = np.array([[0.2 * float(We1[h * CH1:(h + 1) * CH1, 0] @ att1[h])
                     for h in range(H)]], np.float32)
    FC2 = D1 // 128
    Wl2f = Wl2 * w_ln1[None, :]
    Wr2f = Wr2 * w_ln1[None, :]
    wl2T = np.ascontiguousarray(
        Wl2f.T.reshape(FC2, 128, D2).transpose(1, 0, 2)).reshape(128, FC2 * D2)
    wr2T = np.ascontiguousarray(
        Wr2f.T.reshape(FC2, 128, D2).transpose(1, 0, 2)).reshape(128, FC2 * D2)
    attf2 = att2.reshape(D2)
    attm2 = np.zeros((D2, H), np.float32)
    ga2 = np.zeros((D2, H), np.float32)
    for j in range(D2):
        h = j // CH2
        attm2[j, h] = 0.8 * attf2[j]
        ga2[j, h] = 0.2 * attf2[j]
    wa2 = np.array([[0.2 * float(We2[h * CH2:(h + 1) * CH2, 0] @ att2[h])
                     for h in range(H)]], np.float32)
    woutT = np.ascontiguousarray((W_out * w_ln3[None, :]).T).astype(np.float32)

    x16 = x.astype(f16)

    # pack every weight into one [128, WCOLS] slab (row 0's tail columns
    # hold the partition-1 vectors; woutT f32 as f16-pair columns); must
    # mirror WOFF in build_gat.
    WCOLS = 2708 + 2 * CLASSES
    wbig = np.zeros((128, WCOLS), f16)
    wbig[:, 0:512] = np.ascontiguousarray(Wl1.T).astype(f16)
    wbig[:, 512:1024] = np.ascontiguousarray(Wr1.T).astype(f16)
    wbig[:, 1024:1536] = wl2T.astype(f16)
    wbig[:, 1536:2048] = wr2T.astype(f16)
    wbig[:, 2048:2056] = attm1.astype(f16)
    wbig[:, 2056:2058] = gl1.astype(f16)
    wbig[:, 2058:2060] = gr1.astype(f16)
    wbig[:, 2060:2062] = attm2.astype(f16)
    wbig[:, 2062:2064] = ga2.astype(f16)
    wbig[0, 2064:2576] = np.ascontiguousarray(We1.T).astype(f16)[0]
    wbig[0, 2576:2704] = np.ascontiguousarray(We2.T).astype(f16)[0]
    wbig[0, 2704:2706] = wa1.astype(f16)[0]
    wbig[0, 2706:2708] = wa2.astype(f16)[0]
    wbig[:, 2708:] = np.ascontiguousarray(woutT).view(f16)

    RPW = math.ceil(128 / n_cores)
    wbig_pad = np.zeros((n_cores * RPW, WCOLS), f16)
    wbig_pad[:128] = wbig

    in_maps = []
    for k in range(n_cores):
        m = core == k
        sg, dl, dloc, eap = prep_core(cfg, src[m], dst[m], ea2[m], k)
        dloc8 = np.ascontiguousarray(
            dloc.reshape(-1, 128).T).astype(np.uint8)
        # dloc bytes ride in idxw as 8 stripes of 16 rows, viewed as i16
        dloc_i16 = np.concatenate(
            [np.ascontiguousarray(dloc8[16 * r:16 * (r + 1)]).view(np.int16)
             for r in range(8)], axis=1)
        idx_all = np.concatenate([_wrap16(sg), _wrap16(dl), dloc_i16],
                                 axis=1)
        xw = np.concatenate(
            [x16[k * Nc:(k + 1) * Nc].reshape(16, -1),
             wbig_pad[RPW * k:RPW * (k + 1)],
             idx_all.view(f16),
             eap.astype(f16).reshape(16, -1)], axis=1)
        in_maps.append({"xw": np.ascontiguousarray(xw)})
    return cfg, in_maps


_NC_CACHE = {}


def _build_cached(cfg):
    key = tuple(sorted(cfg.items()))
    if key not in _NC_CACHE:
        _NC_CACHE[key] = build_gat(cfg)
    return _NC_CACHE[key]


def kernel(**inputs):
    x = np.asarray(inputs["x"], np.float32)
    ei = np.asarray(inputs["edge_index"])
    ea = np.asarray(inputs["edge_attr"], np.float32)[:, 0]
    weights = tuple(np.asarray(inputs[k], np.float32) for k in
                    ("Wl1", "Wr1", "We1", "att1", "Wl2", "Wr2", "We2", "att2",
                     "w_ln1", "w_ln3", "W_out"))
    cfg, in_maps = make_cfg_and_maps(x, ei, ea, weights)
    nc = _build_cached(cfg)
    res = run_bass_kernel_spmd(nc, in_maps, list(range(cfg["n_cores"])))
    out = np.concatenate([res.results[k]["out"][:cfg["Nc"]]
                          for k in range(cfg["n_cores"])], axis=0)
    return np.asarray(out, np.float32)


if __name__ == "__main__":
    import reference as ref
    inputs = {k: np.asarray(v) for k, v in ref.setup_inputs().items()}
    got = kernel(**inputs)
    exp = np.asarray(ref.reference(**inputs))
    rel = np.abs(got - exp).max() / np.abs(exp).max()
    print(f"Relative error: {rel:.3e}")
